# revision 42
# baseline (speedup 1.0000x reference)
"""Trainium2 Bass kernel for nn_LocalAttentionBlock (MQA local attention, window=1024).

Sharding: 8 cores = 2 batches x 4 time-chunks of 1024 queries. Window=1024 means
each 1024-query chunk only needs the 2048 preceding tokens of x for K/V -> no
collectives; each core computes its output rows independently.

v3 layout (bf16 matmul inputs, fp32 PSUM accumulation, fp32 RoPE):
  - x, Wq, Wk, Wv, Wf converted to bf16 on host (halves DMA, full-rate PE
    transposes); weights prepacked so every DMA is contiguous.
  - Phase A streams x in 512-token groups: DMA -> PE-transpose -> xT (bf16);
    the K/V projection chunk for group g is issued after the transposes of
    group g+1 so its xT copies are guaranteed done (software pipeline).
  - RoPE (5 ops, reads the projection PSUM directly, f32 tables):
      u[0:32] = ps[32:64]*(-sin); u[32:64] = ps[0:32]*(+sin)   (partition
      swap is legal because the shifted operand lives in PSUM)
      t = ps[0:64]*[cos;cos]; dst[0:64] = t+u; dst[64:128] = copy(ps[64:128])
  - Per head: qT = Wq_h.T @ xT_hi (+RoPE); logits TRANSPOSED [s, q] with the
    kT s-block stationary; softmax without max-subtraction; band mask applied
    multiplicatively post-exp on the two partial diagonal blocks (gpsimd).
  - PV: probs [s,q] 128-col block stationary, rhs = [v | 1] bf16 -> PSUM
    [q, 129] gives numerator AND denominator in one accumulation pass.
    PV groups go through a FIFO; one is emitted before the logits of every
    even st, keeping a ~5-group cross-head backlog of ready PE work that
    absorbs the exp latency (head h's PV tail runs inside head h+1's early
    logits region). The q projection itself is pipelined one head ahead
    (issued at the previous head's st12/st14) so the RoPE chain never gates
    the first logits of a head.
  - group-3 K/V projection is deferred from phase A into head 0's early
    logits region for the same reason.
  - zero-padded history of chunk 0 contributes exp(0)=1 per padded in-band
    key; corrected by subtracting a host-computed count from the denominator.
  - encoded scaled by 1/den, PE-transposed into an SBUF-resident encT (no
    DRAM roundtrip), final projection with prefetched bf16 Wf + bias.
"""

import math
import os
from contextlib import ExitStack

import numpy as np
import ml_dtypes

import concourse.bass as bass
from concourse import bacc
import concourse.mybir as mybir
import concourse.tile as tile
from concourse.bass_utils import run_bass_kernel_spmd
from concourse.masks import make_identity

F32 = mybir.dt.float32
BF16 = mybir.dt.bfloat16

B, T, W, NH, HD, WIN = 2, 4096, 2048, 16, 128, 1024
TQ, TKV = 1024, 2048
NQT = TQ // 128          # 8 query tiles
NST = TKV // 128         # 16 key tiles
NKT = W // 128           # 16 contraction tiles over width
SCALE = float(HD) ** -0.5
NB = 9                   # band blocks per query tile


def build_program():
    nc = bacc.Bacc(None, target_bir_lowering=False)
    x_kv = nc.declare_dram_parameter("x_kv", [TKV, W], BF16, isOutput=False)
    wq = nc.declare_dram_parameter("wq", [NH * 128, NKT * 128], BF16, isOutput=False)
    wk = nc.declare_dram_parameter("wk", [128, NKT * 128], BF16, isOutput=False)
    wv = nc.declare_dram_parameter("wv", [128, NKT * 128], BF16, isOutput=False)
    wf = nc.declare_dram_parameter("wf", [W, W], BF16, isOutput=False)
    bias = nc.declare_dram_parameter("bias", [1, W], F32, isOutput=False)
    ccf_t = nc.declare_dram_parameter("ccf_t", [64, TKV], F32, isOutput=False)
    snsf_t = nc.declare_dram_parameter("snsf_t", [64, TKV], F32, isOutput=False)
    m0 = nc.declare_dram_parameter("m0", [128, 128], BF16, isOutput=False)
    m8 = nc.declare_dram_parameter("m8", [128, 128], BF16, isOutput=False)
    invc = nc.declare_dram_parameter("invc", [128, NQT], F32, isOutput=False)
    out = nc.declare_dram_parameter("out", [TQ, W], F32, isOutput=True)

    with tile.TileContext(nc) as tc, ExitStack() as ctx:
        singles = ctx.enter_context(tc.tile_pool(name="singles", bufs=1))
        ident_f = singles.tile([128, 128], F32)
        make_identity(nc, ident_f)
        ident_b = singles.tile([128, 128], BF16)
        nc.vector.tensor_copy(ident_b, ident_f)
        # x rows for the first two groups go on the DMA queue FIRST so the
        # transpose stream starts immediately; tables follow.
        xrowp_cm = tc.tile_pool(name="xrow", bufs=8, side="right")
        xrow_p = xrowp_cm.__enter__()
        xrows = {}

        def _load_xrow(tt):
            r = xrow_p.tile([128, W], BF16, tag="xrow", name=f"xrow{tt}")
            nc.sync.dma_start(out=r, in_=x_kv[tt * 128:(tt + 1) * 128, :])
            xrows[tt] = r

        for tt in range(8):
            _load_xrow(tt)
        wk_sb = singles.tile([128, NKT * 128], BF16)
        nc.sync.dma_start(out=wk_sb, in_=wk[:, :])
        wv_sb = singles.tile([128, NKT * 128], BF16)
        nc.sync.dma_start(out=wv_sb, in_=wv[:, :])
        ccf_sb = singles.tile([64, TKV], F32)
        nc.sync.dma_start(out=ccf_sb, in_=ccf_t[:, :])
        snsf_sb = singles.tile([64, TKV], F32)
        nc.sync.dma_start(out=snsf_sb, in_=snsf_t[:, :])
        m0_sb = singles.tile([128, 128], BF16)
        nc.sync.dma_start(out=m0_sb, in_=m0[:, :])
        m8_sb = singles.tile([128, 128], BF16)
        nc.sync.dma_start(out=m8_sb, in_=m8[:, :])
        invc_sb = singles.tile([128, NQT], F32)
        nc.sync.dma_start(out=invc_sb, in_=invc[:, :])

        def _rope(dst, ps, c0, tf, u):
            """dst (bf16 SBUF, [128,512] slice) = rope of ps (f32 PSUM
            [128,512]); tables at columns c0:c0+512. tf/u: [64,512] f32.
            The partition-swapped operands read PSUM (legal); all-SBUF
            partition-shifted reads are rejected by the BIR verifier."""
            ccc = ccf_sb[:, c0:c0 + 512]
            snc = snsf_sb[:, c0:c0 + 512]
            nc.scalar.copy(dst[64:128, :], ps[64:128, :])
            nc.vector.tensor_mul(u[0:32, :], ps[32:64, :], snc[0:32, :])
            nc.vector.tensor_mul(u[32:64, :], ps[0:32, :], snc[32:64, :])
            nc.vector.tensor_mul(tf, ps[0:64, :], ccc)
            nc.vector.tensor_add(dst[0:64, :], tf, u)

        # Persistent across phases
        xthp_cm = tc.tile_pool(name="xthp", bufs=NKT)
        xthp = xthp_cm.__enter__()
        kvp_cm = tc.tile_pool(name="kvp", bufs=1)
        kvp = kvp_cm.__enter__()
        xtlp_cm = tc.tile_pool(name="xtlp", bufs=NKT)
        xtlp = xtlp_cm.__enter__()

        xT_lo = []  # xT_lo[kt] = [128 w, 1024 t] (t in [0,1024))
        xT_hi = []  # t in [1024, 2048)
        for kt in range(NKT):
            xT_lo.append(xtlp.tile([128, TQ], BF16, tag="big", name=f"xtlo{kt}"))
            xT_hi.append(xthp.tile([128, TQ], BF16, tag="xth", name=f"xthi{kt}"))
        kT = kvp.tile([128, TKV], BF16, tag="kT")
        v_aug = []
        for st in range(NST):
            va = kvp.tile([128, 130], BF16, tag=f"vaug{st}", name=f"vaug{st}")
            nc.vector.memset(va[:, 128:129], 1.0)
            v_aug.append(va)

        # ---- Phase A: stream x in 512-token groups (K/V for groups 0-2
        # here, group 3 deferred into phase B head 0) ----
        with tc.tile_pool(name="xtps", bufs=4, space="PSUM") as xtps, \
             tc.tile_pool(name="kps", bufs=2, space="PSUM") as kps, \
             tc.tile_pool(name="vps", bufs=1, space="PSUM") as vps, \
             tc.tile_pool(name="vtps", bufs=1, space="PSUM") as vtps, \
             tc.tile_pool(name="kvtmp", bufs=2) as kvtmp_p, \
             tc.tile_pool(name="ropea", bufs=2) as ropea_p:

            # PE warm-up: the first ~4us are x-DMA-bound with an idle PE;
            # a burst of dummy transposes keeps the array busy so the real
            # transpose stream starts at full clock (p-state/HAM warm).
            wu = vtps.tile([128, 512], BF16, tag="vt", name="warmup")
            for i in range(72):
                nc.tensor.transpose(wu[:, (i % 4) * 128:(i % 4 + 1) * 128],
                                    ident_b, ident_b)

            def kv_block(g):
                # K/V projection + K-RoPE + V transpose for group g; issued
                # one group behind the transposes so the xT copies are done.
                half, col = divmod(g * 512, TQ)
                dst_list = xT_lo if half == 0 else xT_hi
                ps_k = kps.tile([128, 512], F32, tag="pk")
                ps_v = vps.tile([128, 512], F32, tag="pv")
                for kt in range(NKT):
                    nc.tensor.matmul(ps_k, wk_sb[:, kt * 128:(kt + 1) * 128],
                                     dst_list[kt][:, col:col + 512],
                                     start=(kt == 0), stop=(kt == NKT - 1))
                for kt in range(NKT):
                    nc.tensor.matmul(ps_v, wv_sb[:, kt * 128:(kt + 1) * 128],
                                     dst_list[kt][:, col:col + 512],
                                     start=(kt == 0), stop=(kt == NKT - 1))
                c0 = g * 512
                tf = ropea_p.tile([64, 512], F32, tag="tf")
                u = ropea_p.tile([64, 512], F32, tag="u")
                _rope(kT[:, c0:c0 + 512], ps_k, c0, tf, u)
                vb = kvtmp_p.tile([128, 512], BF16, tag="vb")
                nc.vector.tensor_copy(vb, ps_v)
                ps_t = vtps.tile([128, 512], BF16, tag="vt")
                for j in range(4):
                    nc.tensor.transpose(ps_t[:, j * 128:(j + 1) * 128],
                                        vb[:, j * 128:(j + 1) * 128], ident_b)
                for j in range(4):
                    st = g * 4 + j
                    eng = nc.scalar.copy if j % 2 == 0 else nc.vector.tensor_copy
                    eng(v_aug[st][:, 0:128], ps_t[:, j * 128:(j + 1) * 128])

            for g in range(4):
                if g < 2:
                    for j in range(4):
                        _load_xrow((g + 2) * 4 + j)
                rows = [xrows[g * 4 + j] for j in range(4)]
                half, col = divmod(g * 512, TQ)
                dst_list = xT_lo if half == 0 else xT_hi
                if g == 0:
                    # first group: row-pair-major so PE starts when x row 1
                    # lands instead of waiting for row 3
                    for j0 in (0, 2):
                        for kt in range(NKT):
                            ps = xtps.tile([128, 512], BF16, tag="xt")
                            for j in (j0, j0 + 1):
                                nc.tensor.transpose(
                                    ps[:, (j - j0) * 128:(j - j0 + 1) * 128],
                                    rows[j][:, kt * 128:(kt + 1) * 128], ident_b)
                            eng = (nc.scalar.copy if kt % 2 == 0
                                   else nc.vector.tensor_copy)
                            eng(dst_list[kt][:, col + j0 * 128:col + (j0 + 2) * 128],
                                ps[:, 0:256])
                else:
                    for kt in range(NKT):
                        ps = xtps.tile([128, 512], BF16, tag="xt")
                        for j in range(4):
                            nc.tensor.transpose(
                                ps[:, j * 128:(j + 1) * 128],
                                rows[j][:, kt * 128:(kt + 1) * 128], ident_b)
                        # alternate copy engine to halve the copy chain latency
                        eng = nc.scalar.copy if kt % 2 == 0 else nc.vector.tensor_copy
                        eng(dst_list[kt][:, col:col + 512], ps)
                if g >= 1:
                    kv_block(g - 1)
        xrowp_cm.__exit__(None, None, None)
        xtlp_cm.__exit__(None, None, None)

        wfp_cm = tc.tile_pool(name="wfp", bufs=1, side="right")
        wfp = wfp_cm.__enter__()
        wf_sb = []
        for kt in range(NKT):
            wt = wfp.tile([128, W], BF16, tag=f"wf{kt}", name=f"wf{kt}")
            nc.sync.dma_start(out=wt, in_=wf[kt * 128:(kt + 1) * 128, :])
            wf_sb.append(wt)
        encp_cm = tc.tile_pool(name="encp", bufs=1, side="right")
        encp = encp_cm.__enter__()
        encT = []
        for h in range(NH):
            encT.append(encp.tile([128, TQ], BF16, tag=f"enc{h}", name=f"encT{h}"))

        # ---- Phase B: per-head attention ----
        with tc.tile_pool(name="wqp", bufs=2) as wq_p, \
             tc.tile_pool(name="qtp", bufs=2) as qt_p, \
             tc.tile_pool(name="prp", bufs=28) as pr_p, \
             tc.tile_pool(name="ropeb", bufs=1) as ropeb_p, \
             tc.tile_pool(name="encsp", bufs=4) as encs_p, \
             tc.tile_pool(name="dnp", bufs=8) as dn_p, \
             tc.tile_pool(name="qps", bufs=2, space="PSUM") as qps, \
             tc.tile_pool(name="lgps", bufs=3, space="PSUM") as lgps, \
             tc.tile_pool(name="encps", bufs=2, space="PSUM") as encps, \
             tc.tile_pool(name="etps", bufs=1, space="PSUM") as etps:
            probs_store = {}   # (h, st) -> (qlo, [([128,512] bf16 tile, cw)])
            etp_store = {}     # h -> PSUM transpose-staging tile
            pv_pending = []    # FIFO of ready-but-unissued (h, qt) PV groups

            def emit_kv3():
                # K/V projection + K-RoPE + V transpose for token group 3,
                # deferred from phase A (uses phase-B pools).
                col, c0 = 512, 1536  # group 3 = cols 512:1024 of xT_hi
                ps_k = lgps.tile([128, 512], F32, tag="lg")
                ps_v = lgps.tile([128, 512], F32, tag="lg")
                for kt in range(NKT):
                    nc.tensor.matmul(ps_k, wk_sb[:, kt * 128:(kt + 1) * 128],
                                     xT_hi[kt][:, col:col + 512],
                                     start=(kt == 0), stop=(kt == NKT - 1))
                for kt in range(NKT):
                    nc.tensor.matmul(ps_v, wv_sb[:, kt * 128:(kt + 1) * 128],
                                     xT_hi[kt][:, col:col + 512],
                                     start=(kt == 0), stop=(kt == NKT - 1))
                tf = ropeb_p.tile([64, 512], F32, tag="tf")
                u = ropeb_p.tile([64, 512], F32, tag="u")
                _rope(kT[:, c0:c0 + 512], ps_k, c0, tf, u)
                vb = pr_p.tile([128, 512], BF16, tag="pr", name="kv3vb")
                nc.vector.tensor_copy(vb, ps_v)
                etp3 = etps.tile([128, 1024], BF16, tag="et", name="etpkv3")
                for j in range(4):
                    nc.tensor.transpose(etp3[:, j * 128:(j + 1) * 128],
                                        vb[:, j * 128:(j + 1) * 128], ident_b)
                for j in range(4):
                    st2 = 12 + j
                    eng = nc.scalar.copy if j % 2 == 0 else nc.vector.tensor_copy
                    eng(v_aug[st2][:, 0:128], etp3[:, j * 128:(j + 1) * 128])

            def emit_pv(use_qps=False):
                h2, qt = pv_pending.pop(0)
                if use_qps:  # reuse the idle q-proj bank (same tag/shape)
                    ps_q_full = qps.tile([128, 512], F32, tag="q", name=f"pvq{h2}_{qt}")
                    ps_e = ps_q_full[:, 0:129]
                else:
                    ps_e = encps.tile([128, 129], F32, tag="enc")
                for d in range(NB):
                    st2 = qt + d
                    qlo2, chunks2 = probs_store[(h2, st2)]
                    off = (qt - qlo2) * 128
                    pc2, _ = chunks2[off // 512]
                    o = off % 512
                    nc.tensor.matmul(ps_e, pc2[:, o:o + 128], v_aug[st2][:, 0:129],
                                     start=(d == 0), stop=(d == NB - 1))
                den = dn_p.tile([128, 1], F32, tag="den")
                nc.vector.tensor_sub(den, ps_e[:, 128:129], invc_sb[:, qt:qt + 1])
                rec = dn_p.tile([128, 1], F32, tag="rec")
                nc.vector.reciprocal(rec, den)
                enc_s = encs_p.tile([128, 128], BF16, tag="encs")
                nc.vector.tensor_scalar_mul(enc_s, ps_e[:, 0:128], rec)
                if qt == 0:
                    etp_store[h2] = etps.tile([128, 1024], BF16, tag="et",
                                              name=f"etp{h2}")
                nc.tensor.transpose(etp_store[h2][:, qt * 128:(qt + 1) * 128],
                                    enc_s, ident_b)
                if qt == NQT - 1:
                    nc.vector.tensor_copy(encT[h2], etp_store[h2])

            wq_store = {}
            qt_store = {}

            def load_wq(h2):
                wq_h = wq_p.tile([128, NKT * 128], BF16, tag="wqh",
                                 name=f"wq{h2}")
                nc.sync.dma_start(out=wq_h, in_=wq[h2 * 128:(h2 + 1) * 128, :])
                wq_store[h2] = wq_h

            def qproj_half(h2, half):
                # q projection + RoPE for one 512-token half of head h2;
                # pipelined into head h2-1's late st loop so the rope chain
                # never gates head h2's first logits.
                if half == 0:
                    qt_store[h2] = qt_p.tile([128, TQ], BF16, tag="qT",
                                             name=f"qT{h2}")
                qT = qt_store[h2]
                wq_h = wq_store[h2]
                ps_q = qps.tile([128, 512], F32, tag="q")
                for kt in range(NKT):
                    nc.tensor.matmul(ps_q, wq_h[:, kt * 128:(kt + 1) * 128],
                                     xT_hi[kt][:, half * 512:(half + 1) * 512],
                                     start=(kt == 0), stop=(kt == NKT - 1))
                tf = ropeb_p.tile([64, 512], F32, tag="tf")
                u = ropeb_p.tile([64, 512], F32, tag="u")
                _rope(qT[:, half * 512:(half + 1) * 512], ps_q,
                      TQ + half * 512, tf, u)

            load_wq(0)
            qproj_half(0, 0)
            qproj_half(0, 1)
            emit_kv3()
            for h in range(NH):
                qT = qt_store[h]
                for st in range(NST):
                    # emit PV work BEFORE the logits matmul of even sts: a
                    # stalled logits MM would block the PE FIFO, so the ready
                    # PV backlog must be ahead of it in program order
                    if st % 2 == 0 and pv_pending:
                        emit_pv()
                    if h + 1 < NH:
                        if st == 10:
                            load_wq(h + 1)
                        elif st == 12:
                            qproj_half(h + 1, 0)
                        elif st == 14:
                            qproj_half(h + 1, 1)
                    qlo = max(0, st - 8)
                    qhi = min(NQT - 1, st)
                    wst = (qhi - qlo + 1) * 128
                    chunks = []
                    for cc0 in range(0, wst, 512):
                        cw = min(512, wst - cc0)
                        ps_l = lgps.tile([128, 512], F32, tag="lg")
                        nc.tensor.matmul(ps_l[:, :cw], kT[:, st * 128:(st + 1) * 128],
                                         qT[:, qlo * 128 + cc0: qlo * 128 + cc0 + cw],
                                         start=True, stop=True)
                        pc = pr_p.tile([128, 512], BF16, tag="pr",
                                       name=f"pr{h}_{st}_{cc0}")
                        nc.scalar.activation(pc[:, :cw], ps_l[:, :cw],
                                             mybir.ActivationFunctionType.Exp,
                                             scale=SCALE)
                        chunks.append((pc, cw))
                    probs_store[(h, st)] = (qlo, chunks)
                    # partial diagonal masks
                    if qhi == st:  # window-partial block: cols of qt==st
                        off = (st - qlo) * 128
                        pc, _ = chunks[off // 512]
                        o = off % 512
                        nc.gpsimd.tensor_mul(pc[:, o:o + 128], pc[:, o:o + 128], m0_sb)
                    if qlo == st - 8:  # causal-diagonal block: cols of qt==st-8
                        pc, _ = chunks[0]
                        nc.gpsimd.tensor_mul(pc[:, 0:128], pc[:, 0:128], m8_sb)

                    if st >= 8:
                        pv_pending.append((h, st - 8))
            flush_i = 0
            while pv_pending:
                # q-proj PSUM banks are idle by now; rotating them in gives
                # the drain 3 in-flight PV buffers instead of 2
                emit_pv(use_qps=bool(flush_i % 2))
                flush_i += 1

        kvp_cm.__exit__(None, None, None)
        xthp_cm.__exit__(None, None, None)

        # ---- Phase C: final projection out = encT.T @ Wf + bias ----
        with tc.tile_pool(name="bp", bufs=1) as b_p, \
             tc.tile_pool(name="orow", bufs=2) as orow_p, \
             tc.tile_pool(name="fps", bufs=4, space="PSUM") as fps:
            bias_rep = b_p.tile([128, W], F32, tag="bias")
            nc.sync.dma_start(out=bias_rep, in_=bias[:, :].to_broadcast([128, W]))
            for tt in range(NQT):
                ot = orow_p.tile([128, W], F32, tag="orow")
                last = tt == NQT - 1
                # last tile streams out in 256-col chunks to shorten the tail
                ocw = 256 if last else 512
                for oc in range(W // ocw):
                    c0 = oc * ocw
                    ps = fps.tile([128, 512], F32, tag="f")
                    for kt in range(NKT):
                        nc.tensor.matmul(ps[:, :ocw], encT[kt][:, tt * 128:(tt + 1) * 128],
                                         wf_sb[kt][:, c0:c0 + ocw],
                                         start=(kt == 0), stop=(kt == NKT - 1))
                    nc.vector.tensor_add(ot[:, c0:c0 + ocw], ps[:, :ocw],
                                         bias_rep[:, c0:c0 + ocw])
                    if last:
                        nc.sync.dma_start(
                            out=out[tt * 128:(tt + 1) * 128, c0:c0 + ocw],
                            in_=ot[:, c0:c0 + ocw])
                if not last:
                    nc.sync.dma_start(out=out[tt * 128:(tt + 1) * 128, :], in_=ot)

        encp_cm.__exit__(None, None, None)
        wfp_cm.__exit__(None, None, None)
    nc.finalize()
    return nc


_NC = None


def _get_nc():
    global _NC
    if _NC is None:
        _NC = build_program()
    return _NC


def make_in_maps(x, Wq, Wk, Wv, Wf, bf, segment_pos):
    bf16 = ml_dtypes.bfloat16
    x = np.asarray(x, np.float32)
    r = np.arange(128)
    m0_h = (r[:, None] > r[None, :]).astype(bf16)   # valid jj > r
    m8_h = (r[:, None] <= r[None, :]).astype(bf16)  # valid jj <= r
    inv_ts = (10000.0 ** (-2.0 * np.arange(32, dtype=np.float32) / 64.0))
    # Prepack so every weight DMA is contiguous:
    #   wq[h*128+p, kt*128+c] = Wq[kt*128+p, h*128+c]
    #   wk[p, kt*128+c]       = Wk[kt*128+p, c]
    wq_b = np.ascontiguousarray(
        np.asarray(Wq, np.float32).reshape(NKT, 128, NH, 128)
        .transpose(2, 1, 0, 3).reshape(NH * 128, NKT * 128)).astype(bf16)
    wk_b = np.ascontiguousarray(
        np.asarray(Wk, np.float32).reshape(NKT, 128, HD)
        .transpose(1, 0, 2).reshape(128, NKT * HD)).astype(bf16)
    wv_b = np.ascontiguousarray(
        np.asarray(Wv, np.float32).reshape(NKT, 128, HD)
        .transpose(1, 0, 2).reshape(128, NKT * HD)).astype(bf16)
    wf_b = np.asarray(Wf, np.float32).astype(bf16)
    bias_h = np.asarray(bf, np.float32).reshape(1, W)
    in_maps = []
    for core in range(8):
        b, qc = core // 4, core % 4
        if qc == 0:
            x_kv = np.concatenate([np.zeros((WIN, W), np.float32), x[b, :TQ]], 0)
            invc_h = np.maximum(0, (WIN - 1) - np.arange(TQ)).astype(np.float32)
        else:
            x_kv = x[b, (qc - 1) * TQ:(qc + 1) * TQ]
            invc_h = np.zeros(TQ, np.float32)
        pos_kv = ((qc - 1) * TQ + np.arange(TKV)).astype(np.float32)
        sinu = pos_kv[None, :] * inv_ts[:, None]        # [32, TKV]
        cos_h = np.cos(sinu).astype(np.float32)
        sin_h = np.sin(sinu).astype(np.float32)
        ccf_h = np.concatenate([cos_h, cos_h], 0)       # [64, TKV]
        snsf_h = np.concatenate([-sin_h, sin_h], 0)     # [64, TKV]
        in_maps.append({
            "x_kv": np.ascontiguousarray(x_kv).astype(bf16),
            "wq": wq_b, "wk": wk_b, "wv": wv_b, "wf": wf_b,
            "bias": bias_h,
            "ccf_t": ccf_h, "snsf_t": snsf_h,
            "m0": m0_h, "m8": m8_h,
            "invc": invc_h.reshape(NQT, 128).T.copy(),
        })
    return in_maps


def kernel(x, Wq, Wk, Wv, Wf, bf, segment_pos, _trace=False):
    nc = _get_nc()
    in_maps = make_in_maps(x, Wq, Wk, Wv, Wf, bf, segment_pos)
    res = run_bass_kernel_spmd(nc, in_maps, list(range(8)), trace=_trace)
    outs = res.results
    full = np.zeros((B, T, W), np.float32)
    for core in range(8):
        b, qc = core // 4, core % 4
        full[b, qc * TQ:(qc + 1) * TQ] = outs[core]["out"]
    if _trace:
        return full, res
    return full


# revision 63
# speedup vs baseline: 1.0206x; 1.0206x over previous
"""Trainium2 Bass kernel for nn_LocalAttentionBlock (MQA local attention, window=1024).

Sharding: 8 cores = 2 batches x 4 time-chunks of 1024 queries. Window=1024 means
each 1024-query chunk only needs the 2048 preceding tokens of x for K/V -> no
collectives; each core computes its output rows independently.

v3 layout (bf16 matmul inputs, fp32 PSUM accumulation, fp32 RoPE):
  - x, Wq, Wk, Wv, Wf converted to bf16 on host (halves DMA, full-rate PE
    transposes); weights prepacked so every DMA is contiguous.
  - Phase A streams x in 512-token groups: DMA -> PE-transpose -> xT (bf16);
    the K/V projection chunk for group g is issued after the transposes of
    group g+1 so its xT copies are guaranteed done (software pipeline).
  - RoPE (5 ops, reads the projection PSUM directly, f32 tables):
      u[0:32] = ps[32:64]*(-sin); u[32:64] = ps[0:32]*(+sin)   (partition
      swap is legal because the shifted operand lives in PSUM)
      t = ps[0:64]*[cos;cos]; dst[0:64] = t+u; dst[64:128] = copy(ps[64:128])
  - Per head: qT = Wq_h.T @ xT_hi (+RoPE); logits TRANSPOSED [s, q] with the
    kT s-block stationary; softmax without max-subtraction; band mask applied
    multiplicatively post-exp on the two partial diagonal blocks (gpsimd).
  - PV: probs [s,q] 128-col block stationary, rhs = [v | 1] bf16 -> PSUM
    [q, 129] gives numerator AND denominator in one accumulation pass.
    PV groups go through a FIFO; one is emitted before the logits of every
    even st, keeping a ~5-group cross-head backlog of ready PE work that
    absorbs the exp latency (head h's PV tail runs inside head h+1's early
    logits region). The q projection itself is pipelined one head ahead
    (halves issued at the previous head's st7/st9) so the RoPE chain never
    gates the first logits of a head.
  - group-3 K/V projection is deferred from phase A into head 0's early
    logits region for the same reason.
  - zero-padded history of chunk 0 contributes exp(0)=1 per padded in-band
    key; corrected by subtracting a host-computed count from the denominator.
  - encoded scaled by 1/den, PE-transposed into an SBUF-resident encT (no
    DRAM roundtrip), final projection with prefetched bf16 Wf + bias.
"""

import math
import os
from contextlib import ExitStack

import numpy as np
import ml_dtypes

import concourse.bass as bass
from concourse import bacc
import concourse.mybir as mybir
import concourse.tile as tile
from concourse.bass_utils import run_bass_kernel_spmd
from concourse.masks import make_identity

F32 = mybir.dt.float32
BF16 = mybir.dt.bfloat16

B, T, W, NH, HD, WIN = 2, 4096, 2048, 16, 128, 1024
TQ, TKV = 1024, 2048
NQT = TQ // 128          # 8 query tiles
NST = TKV // 128         # 16 key tiles
NKT = W // 128           # 16 contraction tiles over width
SCALE = float(HD) ** -0.5
NB = 9                   # band blocks per query tile


def build_program():
    nc = bacc.Bacc(None, target_bir_lowering=False)
    x_kv = nc.declare_dram_parameter("x_kv", [TKV, W], BF16, isOutput=False)
    wq = nc.declare_dram_parameter("wq", [NH * 128, NKT * 128], BF16, isOutput=False)
    wk = nc.declare_dram_parameter("wk", [128, NKT * 128], BF16, isOutput=False)
    wv = nc.declare_dram_parameter("wv", [128, NKT * 128], BF16, isOutput=False)
    wf = nc.declare_dram_parameter("wf", [W, W], BF16, isOutput=False)
    bias = nc.declare_dram_parameter("bias", [1, W], F32, isOutput=False)
    ccf_t = nc.declare_dram_parameter("ccf_t", [64, TKV], F32, isOutput=False)
    snsf_t = nc.declare_dram_parameter("snsf_t", [64, TKV], F32, isOutput=False)
    m0 = nc.declare_dram_parameter("m0", [128, 128], BF16, isOutput=False)
    m8 = nc.declare_dram_parameter("m8", [128, 128], BF16, isOutput=False)
    invc = nc.declare_dram_parameter("invc", [128, NQT], F32, isOutput=False)
    out = nc.declare_dram_parameter("out", [TQ, W], F32, isOutput=True)

    with tile.TileContext(nc) as tc, ExitStack() as ctx:
        singles = ctx.enter_context(tc.tile_pool(name="singles", bufs=1))
        ident_f = singles.tile([128, 128], F32)
        make_identity(nc, ident_f)
        ident_b = singles.tile([128, 128], BF16)
        nc.vector.tensor_copy(ident_b, ident_f)
        # x rows for the first two groups go on the DMA queue FIRST so the
        # transpose stream starts immediately; tables follow.
        xrowp_cm = tc.tile_pool(name="xrow", bufs=8, side="right")
        xrow_p = xrowp_cm.__enter__()
        xrows = {}

        def _load_xrow(tt):
            r = xrow_p.tile([128, W], BF16, tag="xrow", name=f"xrow{tt}")
            nc.sync.dma_start(out=r, in_=x_kv[tt * 128:(tt + 1) * 128, :])
            xrows[tt] = r

        for tt in range(8):
            _load_xrow(tt)
        wk_sb = singles.tile([128, NKT * 128], BF16)
        nc.sync.dma_start(out=wk_sb, in_=wk[:, :])
        wv_sb = singles.tile([128, NKT * 128], BF16)
        nc.sync.dma_start(out=wv_sb, in_=wv[:, :])
        ccf_sb = singles.tile([64, TKV], F32)
        nc.sync.dma_start(out=ccf_sb, in_=ccf_t[:, :])
        snsf_sb = singles.tile([64, TKV], F32)
        nc.sync.dma_start(out=snsf_sb, in_=snsf_t[:, :])
        m0_sb = singles.tile([128, 128], BF16)
        nc.sync.dma_start(out=m0_sb, in_=m0[:, :])
        m8_sb = singles.tile([128, 128], BF16)
        nc.sync.dma_start(out=m8_sb, in_=m8[:, :])
        invc_sb = singles.tile([128, NQT], F32)
        nc.sync.dma_start(out=invc_sb, in_=invc[:, :])

        def _rope(dst, ps, c0, tf, u):
            """dst (bf16 SBUF, [128,512] slice) = rope of ps (f32 PSUM
            [128,512]); tables at columns c0:c0+512. tf/u: [64,512] f32.
            The partition-swapped operands read PSUM (legal); all-SBUF
            partition-shifted reads are rejected by the BIR verifier."""
            ccc = ccf_sb[:, c0:c0 + 512]
            snc = snsf_sb[:, c0:c0 + 512]
            nc.scalar.copy(dst[64:128, :], ps[64:128, :])
            nc.vector.tensor_mul(u[0:32, :], ps[32:64, :], snc[0:32, :])
            nc.vector.tensor_mul(u[32:64, :], ps[0:32, :], snc[32:64, :])
            nc.vector.tensor_mul(tf, ps[0:64, :], ccc)
            nc.vector.tensor_add(dst[0:64, :], tf, u)

        # Persistent across phases
        xthp_cm = tc.tile_pool(name="xthp", bufs=NKT)
        xthp = xthp_cm.__enter__()
        kvp_cm = tc.tile_pool(name="kvp", bufs=1)
        kvp = kvp_cm.__enter__()
        xtlp_cm = tc.tile_pool(name="xtlp", bufs=NKT)
        xtlp = xtlp_cm.__enter__()

        xT_lo = []  # xT_lo[kt] = [128 w, 1024 t] (t in [0,1024))
        xT_hi = []  # t in [1024, 2048)
        for kt in range(NKT):
            xT_lo.append(xtlp.tile([128, TQ], BF16, tag="big", name=f"xtlo{kt}"))
            xT_hi.append(xthp.tile([128, TQ], BF16, tag="xth", name=f"xthi{kt}"))
        kT = kvp.tile([128, TKV], BF16, tag="kT")
        v_aug = []
        for st in range(NST):
            va = kvp.tile([128, 130], BF16, tag=f"vaug{st}", name=f"vaug{st}")
            nc.vector.memset(va[:, 128:129], 1.0)
            v_aug.append(va)

        # ---- Phase A: stream x in 512-token groups (K/V for groups 0-2
        # here, group 3 deferred into phase B head 0) ----
        with tc.tile_pool(name="xtps", bufs=4, space="PSUM") as xtps, \
             tc.tile_pool(name="kps", bufs=2, space="PSUM") as kps, \
             tc.tile_pool(name="vps", bufs=1, space="PSUM") as vps, \
             tc.tile_pool(name="vtps", bufs=1, space="PSUM") as vtps, \
             tc.tile_pool(name="kvtmp", bufs=2) as kvtmp_p, \
             tc.tile_pool(name="ropea", bufs=2) as ropea_p:

            # PE warm-up: the first ~4us are x-DMA-bound with an idle PE;
            # a burst of dummy transposes keeps the array busy so the real
            # transpose stream starts at full clock (p-state/HAM warm).
            wu = vtps.tile([128, 512], BF16, tag="vt", name="warmup")
            for i in range(72):
                nc.tensor.transpose(wu[:, (i % 4) * 128:(i % 4 + 1) * 128],
                                    ident_b, ident_b)

            def kv_block(g):
                # K/V projection + K-RoPE + V transpose for group g; issued
                # one group behind the transposes so the xT copies are done.
                half, col = divmod(g * 512, TQ)
                dst_list = xT_lo if half == 0 else xT_hi
                ps_k = kps.tile([128, 512], F32, tag="pk")
                ps_v = vps.tile([128, 512], F32, tag="pv")
                for kt in range(NKT):
                    nc.tensor.matmul(ps_k, wk_sb[:, kt * 128:(kt + 1) * 128],
                                     dst_list[kt][:, col:col + 512],
                                     start=(kt == 0), stop=(kt == NKT - 1))
                for kt in range(NKT):
                    nc.tensor.matmul(ps_v, wv_sb[:, kt * 128:(kt + 1) * 128],
                                     dst_list[kt][:, col:col + 512],
                                     start=(kt == 0), stop=(kt == NKT - 1))
                c0 = g * 512
                tf = ropea_p.tile([64, 512], F32, tag="tf")
                u = ropea_p.tile([64, 512], F32, tag="u")
                _rope(kT[:, c0:c0 + 512], ps_k, c0, tf, u)
                vb = kvtmp_p.tile([128, 512], BF16, tag="vb")
                nc.vector.tensor_copy(vb, ps_v)
                ps_t = vtps.tile([128, 512], BF16, tag="vt")
                for j in range(4):
                    nc.tensor.transpose(ps_t[:, j * 128:(j + 1) * 128],
                                        vb[:, j * 128:(j + 1) * 128], ident_b)
                for j in range(4):
                    st = g * 4 + j
                    eng = nc.scalar.copy if j % 2 == 0 else nc.vector.tensor_copy
                    eng(v_aug[st][:, 0:128], ps_t[:, j * 128:(j + 1) * 128])

            for g in range(4):
                if g < 2:
                    for j in range(4):
                        _load_xrow((g + 2) * 4 + j)
                rows = [xrows[g * 4 + j] for j in range(4)]
                half, col = divmod(g * 512, TQ)
                dst_list = xT_lo if half == 0 else xT_hi
                if g == 0:
                    # first group: row-pair-major so PE starts when x row 1
                    # lands instead of waiting for row 3
                    for j0 in (0, 2):
                        for kt in range(NKT):
                            ps = xtps.tile([128, 512], BF16, tag="xt")
                            for j in (j0, j0 + 1):
                                nc.tensor.transpose(
                                    ps[:, (j - j0) * 128:(j - j0 + 1) * 128],
                                    rows[j][:, kt * 128:(kt + 1) * 128], ident_b)
                            eng = (nc.scalar.copy if kt % 2 == 0
                                   else nc.vector.tensor_copy)
                            eng(dst_list[kt][:, col + j0 * 128:col + (j0 + 2) * 128],
                                ps[:, 0:256])
                else:
                    for kt in range(NKT):
                        ps = xtps.tile([128, 512], BF16, tag="xt")
                        for j in range(4):
                            nc.tensor.transpose(
                                ps[:, j * 128:(j + 1) * 128],
                                rows[j][:, kt * 128:(kt + 1) * 128], ident_b)
                        # alternate copy engine to halve the copy chain latency
                        eng = nc.scalar.copy if kt % 2 == 0 else nc.vector.tensor_copy
                        eng(dst_list[kt][:, col:col + 512], ps)
                if g >= 1:
                    kv_block(g - 1)
        xrowp_cm.__exit__(None, None, None)
        xtlp_cm.__exit__(None, None, None)

        wfp_cm = tc.tile_pool(name="wfp", bufs=1, side="right")
        wfp = wfp_cm.__enter__()
        wf_sb = []
        for kt in range(NKT):
            wt = wfp.tile([128, W], BF16, tag=f"wf{kt}", name=f"wf{kt}")
            nc.sync.dma_start(out=wt, in_=wf[kt * 128:(kt + 1) * 128, :])
            wf_sb.append(wt)
        encp_cm = tc.tile_pool(name="encp", bufs=1, side="right")
        encp = encp_cm.__enter__()
        encT = []
        for h in range(NH):
            encT.append(encp.tile([128, TQ], BF16, tag=f"enc{h}", name=f"encT{h}"))

        # ---- Phase B: per-head attention ----
        with tc.tile_pool(name="wqp", bufs=2) as wq_p, \
             tc.tile_pool(name="qtp", bufs=2) as qt_p, \
             tc.tile_pool(name="prp", bufs=28) as pr_p, \
             tc.tile_pool(name="ropeb", bufs=1) as ropeb_p, \
             tc.tile_pool(name="encsp", bufs=4) as encs_p, \
             tc.tile_pool(name="dnp", bufs=8) as dn_p, \
             tc.tile_pool(name="qps", bufs=2, space="PSUM") as qps, \
             tc.tile_pool(name="lgps", bufs=3, space="PSUM") as lgps, \
             tc.tile_pool(name="encps", bufs=2, space="PSUM") as encps, \
             tc.tile_pool(name="etps", bufs=1, space="PSUM") as etps:
            probs_store = {}   # (h, st) -> (qlo, [([128,512] bf16 tile, cw)])
            etp_store = {}     # h -> PSUM transpose-staging tile
            pv_pending = []    # FIFO of ready-but-unissued (h, qt) PV groups

            def emit_kv3():
                # K/V projection + K-RoPE + V transpose for token group 3,
                # deferred from phase A (uses phase-B pools).
                col, c0 = 512, 1536  # group 3 = cols 512:1024 of xT_hi
                ps_k = lgps.tile([128, 512], F32, tag="lg")
                ps_v = lgps.tile([128, 512], F32, tag="lg")
                for kt in range(NKT):
                    nc.tensor.matmul(ps_k, wk_sb[:, kt * 128:(kt + 1) * 128],
                                     xT_hi[kt][:, col:col + 512],
                                     start=(kt == 0), stop=(kt == NKT - 1))
                for kt in range(NKT):
                    nc.tensor.matmul(ps_v, wv_sb[:, kt * 128:(kt + 1) * 128],
                                     xT_hi[kt][:, col:col + 512],
                                     start=(kt == 0), stop=(kt == NKT - 1))
                tf = ropeb_p.tile([64, 512], F32, tag="tf")
                u = ropeb_p.tile([64, 512], F32, tag="u")
                _rope(kT[:, c0:c0 + 512], ps_k, c0, tf, u)
                vb = pr_p.tile([128, 512], BF16, tag="pr", name="kv3vb")
                nc.vector.tensor_copy(vb, ps_v)
                etp3 = etps.tile([128, 1024], BF16, tag="et", name="etpkv3")
                for j in range(4):
                    nc.tensor.transpose(etp3[:, j * 128:(j + 1) * 128],
                                        vb[:, j * 128:(j + 1) * 128], ident_b)
                for j in range(4):
                    st2 = 12 + j
                    eng = nc.scalar.copy if j % 2 == 0 else nc.vector.tensor_copy
                    eng(v_aug[st2][:, 0:128], etp3[:, j * 128:(j + 1) * 128])

            def emit_pv(use_qps=False):
                h2, qt = pv_pending.pop(0)
                if use_qps:  # reuse the idle q-proj bank (same tag/shape)
                    ps_q_full = qps.tile([128, 512], F32, tag="q", name=f"pvq{h2}_{qt}")
                    ps_e = ps_q_full[:, 0:129]
                else:
                    ps_e = encps.tile([128, 129], F32, tag="enc")
                for d in range(NB):
                    st2 = qt + d
                    qlo2, chunks2 = probs_store[(h2, st2)]
                    off = (qt - qlo2) * 128
                    pc2, _ = chunks2[off // 512]
                    o = off % 512
                    nc.tensor.matmul(ps_e, pc2[:, o:o + 128], v_aug[st2][:, 0:129],
                                     start=(d == 0), stop=(d == NB - 1))
                den = dn_p.tile([128, 1], F32, tag="den")
                nc.vector.tensor_sub(den, ps_e[:, 128:129], invc_sb[:, qt:qt + 1])
                rec = dn_p.tile([128, 1], F32, tag="rec")
                nc.vector.reciprocal(rec, den)
                enc_s = encs_p.tile([128, 128], BF16, tag="encs")
                nc.vector.tensor_scalar_mul(enc_s, ps_e[:, 0:128], rec)
                if qt == 0:
                    etp_store[h2] = etps.tile([128, 1024], BF16, tag="et",
                                              name=f"etp{h2}")
                nc.tensor.transpose(etp_store[h2][:, qt * 128:(qt + 1) * 128],
                                    enc_s, ident_b)
                if qt == NQT - 1:
                    nc.vector.tensor_copy(encT[h2], etp_store[h2])

            wq_store = {}
            qt_store = {}

            def load_wq(h2):
                wq_h = wq_p.tile([128, NKT * 128], BF16, tag="wqh",
                                 name=f"wq{h2}")
                nc.sync.dma_start(out=wq_h, in_=wq[h2 * 128:(h2 + 1) * 128, :])
                wq_store[h2] = wq_h

            def qproj_half(h2, half):
                # q projection + RoPE for one 512-token half of head h2;
                # pipelined into head h2-1's st loop (st7/st9) so the rope
                # chain never gates head h2's first logits.
                if half == 0:
                    qt_store[h2] = qt_p.tile([128, TQ], BF16, tag="qT",
                                             name=f"qT{h2}")
                qT = qt_store[h2]
                wq_h = wq_store[h2]
                ps_q = qps.tile([128, 512], F32, tag="q")
                for kt in range(NKT):
                    nc.tensor.matmul(ps_q, wq_h[:, kt * 128:(kt + 1) * 128],
                                     xT_hi[kt][:, half * 512:(half + 1) * 512],
                                     start=(kt == 0), stop=(kt == NKT - 1))
                tf = ropeb_p.tile([64, 512], F32, tag="tf")
                u = ropeb_p.tile([64, 512], F32, tag="u")
                _rope(qT[:, half * 512:(half + 1) * 512], ps_q,
                      TQ + half * 512, tf, u)

            load_wq(0)
            qproj_half(0, 0)
            qproj_half(0, 1)
            emit_kv3()
            for h in range(NH):
                qT = qt_store[h]
                for st in range(NST):
                    # emit PV work BEFORE the logits matmul of even sts: a
                    # stalled logits MM would block the PE FIFO, so the ready
                    # PV backlog must be ahead of it in program order
                    if st % 2 == 0 and pv_pending:
                        emit_pv()
                    if h + 1 < NH:
                        if st == 5:
                            load_wq(h + 1)
                        elif st == 7:
                            qproj_half(h + 1, 0)
                        elif st == 9:
                            qproj_half(h + 1, 1)
                    qlo = max(0, st - 8)
                    qhi = min(NQT - 1, st)
                    wst = (qhi - qlo + 1) * 128
                    chunks = []
                    for cc0 in range(0, wst, 512):
                        cw = min(512, wst - cc0)
                        ps_l = lgps.tile([128, 512], F32, tag="lg")
                        nc.tensor.matmul(ps_l[:, :cw], kT[:, st * 128:(st + 1) * 128],
                                         qT[:, qlo * 128 + cc0: qlo * 128 + cc0 + cw],
                                         start=True, stop=True)
                        pc = pr_p.tile([128, 512], BF16, tag="pr",
                                       name=f"pr{h}_{st}_{cc0}")
                        nc.scalar.activation(pc[:, :cw], ps_l[:, :cw],
                                             mybir.ActivationFunctionType.Exp,
                                             scale=SCALE)
                        chunks.append((pc, cw))
                    probs_store[(h, st)] = (qlo, chunks)
                    # partial diagonal masks
                    if qhi == st:  # window-partial block: cols of qt==st
                        off = (st - qlo) * 128
                        pc, _ = chunks[off // 512]
                        o = off % 512
                        nc.gpsimd.tensor_mul(pc[:, o:o + 128], pc[:, o:o + 128], m0_sb)
                    if qlo == st - 8:  # causal-diagonal block: cols of qt==st-8
                        pc, _ = chunks[0]
                        nc.gpsimd.tensor_mul(pc[:, 0:128], pc[:, 0:128], m8_sb)

                    if st >= 8:
                        pv_pending.append((h, st - 8))
            flush_i = 0
            while pv_pending:
                # q-proj PSUM banks are idle by now; rotating them in gives
                # the drain 3 in-flight PV buffers instead of 2
                emit_pv(use_qps=bool(flush_i % 2))
                flush_i += 1

        kvp_cm.__exit__(None, None, None)
        xthp_cm.__exit__(None, None, None)

        # ---- Phase C: final projection out = encT.T @ Wf + bias ----
        with tc.tile_pool(name="bp", bufs=1) as b_p, \
             tc.tile_pool(name="orow", bufs=2) as orow_p, \
             tc.tile_pool(name="fps", bufs=4, space="PSUM") as fps:
            bias_rep = b_p.tile([128, W], F32, tag="bias")
            nc.sync.dma_start(out=bias_rep, in_=bias[:, :].to_broadcast([128, W]))
            for tt in range(NQT):
                ot = orow_p.tile([128, W], F32, tag="orow")
                last = tt == NQT - 1
                # last tile streams out in 256-col chunks to shorten the tail
                ocw = 256 if last else 512
                for oc in range(W // ocw):
                    c0 = oc * ocw
                    ps = fps.tile([128, 512], F32, tag="f")
                    for kt in range(NKT):
                        nc.tensor.matmul(ps[:, :ocw], encT[kt][:, tt * 128:(tt + 1) * 128],
                                         wf_sb[kt][:, c0:c0 + ocw],
                                         start=(kt == 0), stop=(kt == NKT - 1))
                    nc.vector.tensor_add(ot[:, c0:c0 + ocw], ps[:, :ocw],
                                         bias_rep[:, c0:c0 + ocw])
                    if last:
                        nc.sync.dma_start(
                            out=out[tt * 128:(tt + 1) * 128, c0:c0 + ocw],
                            in_=ot[:, c0:c0 + ocw])
                if not last:
                    nc.sync.dma_start(out=out[tt * 128:(tt + 1) * 128, :], in_=ot)

        encp_cm.__exit__(None, None, None)
        wfp_cm.__exit__(None, None, None)
    nc.finalize()
    return nc


_NC = None


def _get_nc():
    global _NC
    if _NC is None:
        _NC = build_program()
    return _NC


def make_in_maps(x, Wq, Wk, Wv, Wf, bf, segment_pos):
    bf16 = ml_dtypes.bfloat16
    x = np.asarray(x, np.float32)
    r = np.arange(128)
    m0_h = (r[:, None] > r[None, :]).astype(bf16)   # valid jj > r
    m8_h = (r[:, None] <= r[None, :]).astype(bf16)  # valid jj <= r
    inv_ts = (10000.0 ** (-2.0 * np.arange(32, dtype=np.float32) / 64.0))
    # Prepack so every weight DMA is contiguous:
    #   wq[h*128+p, kt*128+c] = Wq[kt*128+p, h*128+c]
    #   wk[p, kt*128+c]       = Wk[kt*128+p, c]
    wq_b = np.ascontiguousarray(
        np.asarray(Wq, np.float32).reshape(NKT, 128, NH, 128)
        .transpose(2, 1, 0, 3).reshape(NH * 128, NKT * 128)).astype(bf16)
    wk_b = np.ascontiguousarray(
        np.asarray(Wk, np.float32).reshape(NKT, 128, HD)
        .transpose(1, 0, 2).reshape(128, NKT * HD)).astype(bf16)
    wv_b = np.ascontiguousarray(
        np.asarray(Wv, np.float32).reshape(NKT, 128, HD)
        .transpose(1, 0, 2).reshape(128, NKT * HD)).astype(bf16)
    wf_b = np.asarray(Wf, np.float32).astype(bf16)
    bias_h = np.asarray(bf, np.float32).reshape(1, W)
    in_maps = []
    for core in range(8):
        b, qc = core // 4, core % 4
        if qc == 0:
            x_kv = np.concatenate([np.zeros((WIN, W), np.float32), x[b, :TQ]], 0)
            invc_h = np.maximum(0, (WIN - 1) - np.arange(TQ)).astype(np.float32)
        else:
            x_kv = x[b, (qc - 1) * TQ:(qc + 1) * TQ]
            invc_h = np.zeros(TQ, np.float32)
        pos_kv = ((qc - 1) * TQ + np.arange(TKV)).astype(np.float32)
        sinu = pos_kv[None, :] * inv_ts[:, None]        # [32, TKV]
        cos_h = np.cos(sinu).astype(np.float32)
        sin_h = np.sin(sinu).astype(np.float32)
        ccf_h = np.concatenate([cos_h, cos_h], 0)       # [64, TKV]
        snsf_h = np.concatenate([-sin_h, sin_h], 0)     # [64, TKV]
        in_maps.append({
            "x_kv": np.ascontiguousarray(x_kv).astype(bf16),
            "wq": wq_b, "wk": wk_b, "wv": wv_b, "wf": wf_b,
            "bias": bias_h,
            "ccf_t": ccf_h, "snsf_t": snsf_h,
            "m0": m0_h, "m8": m8_h,
            "invc": invc_h.reshape(NQT, 128).T.copy(),
        })
    return in_maps


def kernel(x, Wq, Wk, Wv, Wf, bf, segment_pos, _trace=False):
    nc = _get_nc()
    in_maps = make_in_maps(x, Wq, Wk, Wv, Wf, bf, segment_pos)
    res = run_bass_kernel_spmd(nc, in_maps, list(range(8)), trace=_trace)
    outs = res.results
    full = np.zeros((B, T, W), np.float32)
    for core in range(8):
        b, qc = core // 4, core % 4
        full[b, qc * TQ:(qc + 1) * TQ] = outs[core]["out"]
    if _trace:
        return full, res
    return full


# revision 68
# speedup vs baseline: 1.0208x; 1.0001x over previous
"""Trainium2 Bass kernel for nn_LocalAttentionBlock (MQA local attention, window=1024).

Sharding: 8 cores = 2 batches x 4 time-chunks of 1024 queries. Window=1024 means
each 1024-query chunk only needs the 2048 preceding tokens of x for K/V -> no
collectives; each core computes its output rows independently.

v3 layout (bf16 matmul inputs, fp32 PSUM accumulation, fp32 RoPE):
  - x, Wq, Wk, Wv, Wf converted to bf16 on host (halves DMA, full-rate PE
    transposes); weights prepacked so every DMA is contiguous.
  - Phase A streams x in 512-token groups: DMA -> PE-transpose -> xT (bf16);
    the K/V projection chunk for group g is issued after the transposes of
    group g+1 so its xT copies are guaranteed done (software pipeline).
  - RoPE (5 ops, reads the projection PSUM directly, f32 tables):
      u[0:32] = ps[32:64]*(-sin); u[32:64] = ps[0:32]*(+sin)   (partition
      swap is legal because the shifted operand lives in PSUM)
      t = ps[0:64]*[cos;cos]; dst[0:64] = t+u; dst[64:128] = copy(ps[64:128])
  - Per head: qT = Wq_h.T @ xT_hi (+RoPE); logits TRANSPOSED [s, q] with the
    kT s-block stationary; softmax without max-subtraction; band mask applied
    multiplicatively post-exp on the two partial diagonal blocks (gpsimd).
  - PV: probs [s,q] 128-col block stationary, rhs = [v | 1] bf16 -> PSUM
    [q, 129] gives numerator AND denominator in one accumulation pass.
    PV groups go through a FIFO; one is emitted before the logits of every
    even st, keeping a ~5-group cross-head backlog of ready PE work that
    absorbs the exp latency (head h's PV tail runs inside head h+1's early
    logits region). The q projection itself is pipelined one head ahead
    (halves issued at the previous head's st7/st9) so the RoPE chain never
    gates the first logits of a head.
  - group-3 K/V projection is deferred from phase A into head 0's early
    logits region for the same reason.
  - zero-padded history of chunk 0 contributes exp(0)=1 per padded in-band
    key; corrected by subtracting a host-computed count from the denominator.
  - encoded scaled by 1/den, PE-transposed into an SBUF-resident encT (no
    DRAM roundtrip), final projection with prefetched bf16 Wf + bias.
"""

import math
import os
from contextlib import ExitStack

import numpy as np
import ml_dtypes

import concourse.bass as bass
from concourse import bacc
import concourse.mybir as mybir
import concourse.tile as tile
from concourse.bass_utils import run_bass_kernel_spmd
from concourse.masks import make_identity

F32 = mybir.dt.float32
BF16 = mybir.dt.bfloat16

B, T, W, NH, HD, WIN = 2, 4096, 2048, 16, 128, 1024
TQ, TKV = 1024, 2048
NQT = TQ // 128          # 8 query tiles
NST = TKV // 128         # 16 key tiles
NKT = W // 128           # 16 contraction tiles over width
SCALE = float(HD) ** -0.5
NB = 9                   # band blocks per query tile


def build_program():
    nc = bacc.Bacc(None, target_bir_lowering=False)
    x_kv = nc.declare_dram_parameter("x_kv", [TKV, W], BF16, isOutput=False)
    wq = nc.declare_dram_parameter("wq", [NH * 128, NKT * 128], BF16, isOutput=False)
    wk = nc.declare_dram_parameter("wk", [128, NKT * 128], BF16, isOutput=False)
    wv = nc.declare_dram_parameter("wv", [128, NKT * 128], BF16, isOutput=False)
    wf = nc.declare_dram_parameter("wf", [W, W], BF16, isOutput=False)
    bias = nc.declare_dram_parameter("bias", [1, W], F32, isOutput=False)
    ccf_t = nc.declare_dram_parameter("ccf_t", [64, TKV], F32, isOutput=False)
    snsf_t = nc.declare_dram_parameter("snsf_t", [64, TKV], F32, isOutput=False)
    m0 = nc.declare_dram_parameter("m0", [128, 128], BF16, isOutput=False)
    m8 = nc.declare_dram_parameter("m8", [128, 128], BF16, isOutput=False)
    invc = nc.declare_dram_parameter("invc", [128, NQT], F32, isOutput=False)
    out = nc.declare_dram_parameter("out", [TQ, W], F32, isOutput=True)

    with tile.TileContext(nc) as tc, ExitStack() as ctx:
        singles = ctx.enter_context(tc.tile_pool(name="singles", bufs=1))
        ident_f = singles.tile([128, 128], F32)
        make_identity(nc, ident_f)
        ident_b = singles.tile([128, 128], BF16)
        nc.vector.tensor_copy(ident_b, ident_f)
        # x rows for the first two groups go on the DMA queue FIRST so the
        # transpose stream starts immediately; tables follow.
        xrowp_cm = tc.tile_pool(name="xrow", bufs=8, side="right")
        xrow_p = xrowp_cm.__enter__()
        xrows = {}

        def _load_xrow(tt):
            r = xrow_p.tile([128, W], BF16, tag="xrow", name=f"xrow{tt}")
            nc.sync.dma_start(out=r, in_=x_kv[tt * 128:(tt + 1) * 128, :])
            xrows[tt] = r

        for tt in range(8):
            _load_xrow(tt)
        wk_sb = singles.tile([128, NKT * 128], BF16)
        nc.sync.dma_start(out=wk_sb, in_=wk[:, :])
        wv_sb = singles.tile([128, NKT * 128], BF16)
        nc.sync.dma_start(out=wv_sb, in_=wv[:, :])
        ccf_sb = singles.tile([64, TKV], F32)
        nc.sync.dma_start(out=ccf_sb, in_=ccf_t[:, :])
        snsf_sb = singles.tile([64, TKV], F32)
        nc.sync.dma_start(out=snsf_sb, in_=snsf_t[:, :])
        m0_sb = singles.tile([128, 128], BF16)
        nc.sync.dma_start(out=m0_sb, in_=m0[:, :])
        m8_sb = singles.tile([128, 128], BF16)
        nc.sync.dma_start(out=m8_sb, in_=m8[:, :])
        invc_sb = singles.tile([128, NQT], F32)
        nc.sync.dma_start(out=invc_sb, in_=invc[:, :])

        def _rope(dst, ps, c0, tf, u):
            """dst (bf16 SBUF, [128,512] slice) = rope of ps (f32 PSUM
            [128,512]); tables at columns c0:c0+512. tf/u: [64,512] f32.
            The partition-swapped operands read PSUM (legal); all-SBUF
            partition-shifted reads are rejected by the BIR verifier."""
            ccc = ccf_sb[:, c0:c0 + 512]
            snc = snsf_sb[:, c0:c0 + 512]
            nc.scalar.copy(dst[64:128, :], ps[64:128, :])
            nc.vector.tensor_mul(u[0:32, :], ps[32:64, :], snc[0:32, :])
            nc.vector.tensor_mul(u[32:64, :], ps[0:32, :], snc[32:64, :])
            nc.vector.tensor_mul(tf, ps[0:64, :], ccc)
            nc.vector.tensor_add(dst[0:64, :], tf, u)

        # Persistent across phases
        xthp_cm = tc.tile_pool(name="xthp", bufs=NKT)
        xthp = xthp_cm.__enter__()
        kvp_cm = tc.tile_pool(name="kvp", bufs=1)
        kvp = kvp_cm.__enter__()
        xtlp_cm = tc.tile_pool(name="xtlp", bufs=NKT)
        xtlp = xtlp_cm.__enter__()

        xT_lo = []  # xT_lo[kt] = [128 w, 1024 t] (t in [0,1024))
        xT_hi = []  # t in [1024, 2048)
        for kt in range(NKT):
            xT_lo.append(xtlp.tile([128, TQ], BF16, tag="big", name=f"xtlo{kt}"))
            xT_hi.append(xthp.tile([128, TQ], BF16, tag="xth", name=f"xthi{kt}"))
        kT = kvp.tile([128, TKV], BF16, tag="kT")
        v_aug = []
        for st in range(NST):
            va = kvp.tile([128, 130], BF16, tag=f"vaug{st}", name=f"vaug{st}")
            nc.vector.memset(va[:, 128:129], 1.0)
            v_aug.append(va)

        # ---- Phase A: stream x in 512-token groups (K/V for groups 0-2
        # here, group 3 deferred into phase B head 0) ----
        with tc.tile_pool(name="xtps", bufs=4, space="PSUM") as xtps, \
             tc.tile_pool(name="kps", bufs=2, space="PSUM") as kps, \
             tc.tile_pool(name="vps", bufs=1, space="PSUM") as vps, \
             tc.tile_pool(name="vtps", bufs=1, space="PSUM") as vtps, \
             tc.tile_pool(name="kvtmp", bufs=2) as kvtmp_p, \
             tc.tile_pool(name="ropea", bufs=2) as ropea_p:

            # PE warm-up: the first ~4us are x-DMA-bound with an idle PE;
            # a burst of dummy transposes keeps the array busy so the real
            # transpose stream starts at full clock (p-state/HAM warm).
            wu = vtps.tile([128, 512], BF16, tag="vt", name="warmup")
            for i in range(40):
                nc.tensor.transpose(wu[:, (i % 4) * 128:(i % 4 + 1) * 128],
                                    ident_b, ident_b)

            def kv_block(g):
                # K/V projection + K-RoPE + V transpose for group g; issued
                # one group behind the transposes so the xT copies are done.
                half, col = divmod(g * 512, TQ)
                dst_list = xT_lo if half == 0 else xT_hi
                ps_k = kps.tile([128, 512], F32, tag="pk")
                ps_v = vps.tile([128, 512], F32, tag="pv")
                for kt in range(NKT):
                    nc.tensor.matmul(ps_k, wk_sb[:, kt * 128:(kt + 1) * 128],
                                     dst_list[kt][:, col:col + 512],
                                     start=(kt == 0), stop=(kt == NKT - 1))
                for kt in range(NKT):
                    nc.tensor.matmul(ps_v, wv_sb[:, kt * 128:(kt + 1) * 128],
                                     dst_list[kt][:, col:col + 512],
                                     start=(kt == 0), stop=(kt == NKT - 1))
                c0 = g * 512
                tf = ropea_p.tile([64, 512], F32, tag="tf")
                u = ropea_p.tile([64, 512], F32, tag="u")
                _rope(kT[:, c0:c0 + 512], ps_k, c0, tf, u)
                vb = kvtmp_p.tile([128, 512], BF16, tag="vb")
                nc.vector.tensor_copy(vb, ps_v)
                ps_t = vtps.tile([128, 512], BF16, tag="vt")
                for j in range(4):
                    nc.tensor.transpose(ps_t[:, j * 128:(j + 1) * 128],
                                        vb[:, j * 128:(j + 1) * 128], ident_b)
                for j in range(4):
                    st = g * 4 + j
                    eng = nc.scalar.copy if j % 2 == 0 else nc.vector.tensor_copy
                    eng(v_aug[st][:, 0:128], ps_t[:, j * 128:(j + 1) * 128])

            for g in range(4):
                if g < 2:
                    for j in range(4):
                        _load_xrow((g + 2) * 4 + j)
                rows = [xrows[g * 4 + j] for j in range(4)]
                half, col = divmod(g * 512, TQ)
                dst_list = xT_lo if half == 0 else xT_hi
                if g == 0:
                    # first group: row-pair-major so PE starts when x row 1
                    # lands instead of waiting for row 3
                    for j0 in (0, 2):
                        for kt in range(NKT):
                            ps = xtps.tile([128, 512], BF16, tag="xt")
                            for j in (j0, j0 + 1):
                                nc.tensor.transpose(
                                    ps[:, (j - j0) * 128:(j - j0 + 1) * 128],
                                    rows[j][:, kt * 128:(kt + 1) * 128], ident_b)
                            eng = (nc.scalar.copy if kt % 2 == 0
                                   else nc.vector.tensor_copy)
                            eng(dst_list[kt][:, col + j0 * 128:col + (j0 + 2) * 128],
                                ps[:, 0:256])
                else:
                    for kt in range(NKT):
                        ps = xtps.tile([128, 512], BF16, tag="xt")
                        for j in range(4):
                            nc.tensor.transpose(
                                ps[:, j * 128:(j + 1) * 128],
                                rows[j][:, kt * 128:(kt + 1) * 128], ident_b)
                        # alternate copy engine to halve the copy chain latency
                        eng = nc.scalar.copy if kt % 2 == 0 else nc.vector.tensor_copy
                        eng(dst_list[kt][:, col:col + 512], ps)
                if g >= 1:
                    kv_block(g - 1)
        xrowp_cm.__exit__(None, None, None)
        xtlp_cm.__exit__(None, None, None)

        wfp_cm = tc.tile_pool(name="wfp", bufs=1, side="right")
        wfp = wfp_cm.__enter__()
        wf_sb = []
        for kt in range(NKT):
            wt = wfp.tile([128, W], BF16, tag=f"wf{kt}", name=f"wf{kt}")
            nc.sync.dma_start(out=wt, in_=wf[kt * 128:(kt + 1) * 128, :])
            wf_sb.append(wt)
        encp_cm = tc.tile_pool(name="encp", bufs=1, side="right")
        encp = encp_cm.__enter__()
        encT = []
        for h in range(NH):
            encT.append(encp.tile([128, TQ], BF16, tag=f"enc{h}", name=f"encT{h}"))

        # ---- Phase B: per-head attention ----
        with tc.tile_pool(name="wqp", bufs=2) as wq_p, \
             tc.tile_pool(name="qtp", bufs=2) as qt_p, \
             tc.tile_pool(name="prp", bufs=28) as pr_p, \
             tc.tile_pool(name="ropeb", bufs=1) as ropeb_p, \
             tc.tile_pool(name="encsp", bufs=4) as encs_p, \
             tc.tile_pool(name="dnp", bufs=8) as dn_p, \
             tc.tile_pool(name="qps", bufs=2, space="PSUM") as qps, \
             tc.tile_pool(name="lgps", bufs=3, space="PSUM") as lgps, \
             tc.tile_pool(name="encps", bufs=2, space="PSUM") as encps, \
             tc.tile_pool(name="etps", bufs=1, space="PSUM") as etps:
            probs_store = {}   # (h, st) -> (qlo, [([128,512] bf16 tile, cw)])
            etp_store = {}     # h -> PSUM transpose-staging tile
            pv_pending = []    # FIFO of ready-but-unissued (h, qt) PV groups

            def emit_kv3():
                # K/V projection + K-RoPE + V transpose for token group 3,
                # deferred from phase A (uses phase-B pools).
                col, c0 = 512, 1536  # group 3 = cols 512:1024 of xT_hi
                ps_k = lgps.tile([128, 512], F32, tag="lg")
                ps_v = lgps.tile([128, 512], F32, tag="lg")
                for kt in range(NKT):
                    nc.tensor.matmul(ps_k, wk_sb[:, kt * 128:(kt + 1) * 128],
                                     xT_hi[kt][:, col:col + 512],
                                     start=(kt == 0), stop=(kt == NKT - 1))
                for kt in range(NKT):
                    nc.tensor.matmul(ps_v, wv_sb[:, kt * 128:(kt + 1) * 128],
                                     xT_hi[kt][:, col:col + 512],
                                     start=(kt == 0), stop=(kt == NKT - 1))
                tf = ropeb_p.tile([64, 512], F32, tag="tf")
                u = ropeb_p.tile([64, 512], F32, tag="u")
                _rope(kT[:, c0:c0 + 512], ps_k, c0, tf, u)
                vb = pr_p.tile([128, 512], BF16, tag="pr", name="kv3vb")
                nc.vector.tensor_copy(vb, ps_v)
                etp3 = etps.tile([128, 1024], BF16, tag="et", name="etpkv3")
                for j in range(4):
                    nc.tensor.transpose(etp3[:, j * 128:(j + 1) * 128],
                                        vb[:, j * 128:(j + 1) * 128], ident_b)
                for j in range(4):
                    st2 = 12 + j
                    eng = nc.scalar.copy if j % 2 == 0 else nc.vector.tensor_copy
                    eng(v_aug[st2][:, 0:128], etp3[:, j * 128:(j + 1) * 128])

            def emit_pv(use_qps=False):
                h2, qt = pv_pending.pop(0)
                if use_qps:  # reuse the idle q-proj bank (same tag/shape)
                    ps_q_full = qps.tile([128, 512], F32, tag="q", name=f"pvq{h2}_{qt}")
                    ps_e = ps_q_full[:, 0:129]
                else:
                    ps_e = encps.tile([128, 129], F32, tag="enc")
                for d in range(NB):
                    st2 = qt + d
                    qlo2, chunks2 = probs_store[(h2, st2)]
                    off = (qt - qlo2) * 128
                    pc2, _ = chunks2[off // 512]
                    o = off % 512
                    nc.tensor.matmul(ps_e, pc2[:, o:o + 128], v_aug[st2][:, 0:129],
                                     start=(d == 0), stop=(d == NB - 1))
                den = dn_p.tile([128, 1], F32, tag="den")
                nc.vector.tensor_sub(den, ps_e[:, 128:129], invc_sb[:, qt:qt + 1])
                rec = dn_p.tile([128, 1], F32, tag="rec")
                nc.vector.reciprocal(rec, den)
                enc_s = encs_p.tile([128, 128], BF16, tag="encs")
                nc.vector.tensor_scalar_mul(enc_s, ps_e[:, 0:128], rec)
                if qt == 0:
                    etp_store[h2] = etps.tile([128, 1024], BF16, tag="et",
                                              name=f"etp{h2}")
                nc.tensor.transpose(etp_store[h2][:, qt * 128:(qt + 1) * 128],
                                    enc_s, ident_b)
                if qt == NQT - 1:
                    nc.vector.tensor_copy(encT[h2], etp_store[h2])

            wq_store = {}
            qt_store = {}

            def load_wq(h2):
                wq_h = wq_p.tile([128, NKT * 128], BF16, tag="wqh",
                                 name=f"wq{h2}")
                nc.sync.dma_start(out=wq_h, in_=wq[h2 * 128:(h2 + 1) * 128, :])
                wq_store[h2] = wq_h

            def qproj_half(h2, half):
                # q projection + RoPE for one 512-token half of head h2;
                # pipelined into head h2-1's st loop (st7/st9) so the rope
                # chain never gates head h2's first logits.
                if half == 0:
                    qt_store[h2] = qt_p.tile([128, TQ], BF16, tag="qT",
                                             name=f"qT{h2}")
                qT = qt_store[h2]
                wq_h = wq_store[h2]
                ps_q = qps.tile([128, 512], F32, tag="q")
                for kt in range(NKT):
                    nc.tensor.matmul(ps_q, wq_h[:, kt * 128:(kt + 1) * 128],
                                     xT_hi[kt][:, half * 512:(half + 1) * 512],
                                     start=(kt == 0), stop=(kt == NKT - 1))
                tf = ropeb_p.tile([64, 512], F32, tag="tf")
                u = ropeb_p.tile([64, 512], F32, tag="u")
                _rope(qT[:, half * 512:(half + 1) * 512], ps_q,
                      TQ + half * 512, tf, u)

            load_wq(0)
            qproj_half(0, 0)
            qproj_half(0, 1)
            emit_kv3()
            for h in range(NH):
                qT = qt_store[h]
                for st in range(NST):
                    # emit PV work BEFORE the logits matmul of even sts: a
                    # stalled logits MM would block the PE FIFO, so the ready
                    # PV backlog must be ahead of it in program order
                    if st % 2 == 0 and pv_pending:
                        emit_pv()
                    if h + 1 < NH:
                        if st == 5:
                            load_wq(h + 1)
                        elif st == 7:
                            qproj_half(h + 1, 0)
                        elif st == 9:
                            qproj_half(h + 1, 1)
                    qlo = max(0, st - 8)
                    qhi = min(NQT - 1, st)
                    wst = (qhi - qlo + 1) * 128
                    chunks = []
                    for cc0 in range(0, wst, 512):
                        cw = min(512, wst - cc0)
                        ps_l = lgps.tile([128, 512], F32, tag="lg")
                        nc.tensor.matmul(ps_l[:, :cw], kT[:, st * 128:(st + 1) * 128],
                                         qT[:, qlo * 128 + cc0: qlo * 128 + cc0 + cw],
                                         start=True, stop=True)
                        pc = pr_p.tile([128, 512], BF16, tag="pr",
                                       name=f"pr{h}_{st}_{cc0}")
                        nc.scalar.activation(pc[:, :cw], ps_l[:, :cw],
                                             mybir.ActivationFunctionType.Exp,
                                             scale=SCALE)
                        chunks.append((pc, cw))
                    probs_store[(h, st)] = (qlo, chunks)
                    # partial diagonal masks
                    if qhi == st:  # window-partial block: cols of qt==st
                        off = (st - qlo) * 128
                        pc, _ = chunks[off // 512]
                        o = off % 512
                        nc.gpsimd.tensor_mul(pc[:, o:o + 128], pc[:, o:o + 128], m0_sb)
                    if qlo == st - 8:  # causal-diagonal block: cols of qt==st-8
                        pc, _ = chunks[0]
                        nc.gpsimd.tensor_mul(pc[:, 0:128], pc[:, 0:128], m8_sb)

                    if st >= 8:
                        pv_pending.append((h, st - 8))
            flush_i = 0
            while pv_pending:
                # q-proj PSUM banks are idle by now; rotating them in gives
                # the drain 3 in-flight PV buffers instead of 2
                emit_pv(use_qps=bool(flush_i % 2))
                flush_i += 1

        kvp_cm.__exit__(None, None, None)
        xthp_cm.__exit__(None, None, None)

        # ---- Phase C: final projection out = encT.T @ Wf + bias ----
        with tc.tile_pool(name="bp", bufs=1) as b_p, \
             tc.tile_pool(name="orow", bufs=2) as orow_p, \
             tc.tile_pool(name="fps", bufs=4, space="PSUM") as fps:
            bias_rep = b_p.tile([128, W], F32, tag="bias")
            nc.sync.dma_start(out=bias_rep, in_=bias[:, :].to_broadcast([128, W]))
            for tt in range(NQT):
                ot = orow_p.tile([128, W], F32, tag="orow")
                last = tt == NQT - 1
                # last tile streams out in 256-col chunks to shorten the tail
                ocw = 256 if last else 512
                for oc in range(W // ocw):
                    c0 = oc * ocw
                    ps = fps.tile([128, 512], F32, tag="f")
                    for kt in range(NKT):
                        nc.tensor.matmul(ps[:, :ocw], encT[kt][:, tt * 128:(tt + 1) * 128],
                                         wf_sb[kt][:, c0:c0 + ocw],
                                         start=(kt == 0), stop=(kt == NKT - 1))
                    nc.vector.tensor_add(ot[:, c0:c0 + ocw], ps[:, :ocw],
                                         bias_rep[:, c0:c0 + ocw])
                    if last:
                        nc.sync.dma_start(
                            out=out[tt * 128:(tt + 1) * 128, c0:c0 + ocw],
                            in_=ot[:, c0:c0 + ocw])
                if not last:
                    nc.sync.dma_start(out=out[tt * 128:(tt + 1) * 128, :], in_=ot)

        encp_cm.__exit__(None, None, None)
        wfp_cm.__exit__(None, None, None)
    nc.finalize()
    return nc


_NC = None


def _get_nc():
    global _NC
    if _NC is None:
        _NC = build_program()
    return _NC


def make_in_maps(x, Wq, Wk, Wv, Wf, bf, segment_pos):
    bf16 = ml_dtypes.bfloat16
    x = np.asarray(x, np.float32)
    r = np.arange(128)
    m0_h = (r[:, None] > r[None, :]).astype(bf16)   # valid jj > r
    m8_h = (r[:, None] <= r[None, :]).astype(bf16)  # valid jj <= r
    inv_ts = (10000.0 ** (-2.0 * np.arange(32, dtype=np.float32) / 64.0))
    # Prepack so every weight DMA is contiguous:
    #   wq[h*128+p, kt*128+c] = Wq[kt*128+p, h*128+c]
    #   wk[p, kt*128+c]       = Wk[kt*128+p, c]
    wq_b = np.ascontiguousarray(
        np.asarray(Wq, np.float32).reshape(NKT, 128, NH, 128)
        .transpose(2, 1, 0, 3).reshape(NH * 128, NKT * 128)).astype(bf16)
    wk_b = np.ascontiguousarray(
        np.asarray(Wk, np.float32).reshape(NKT, 128, HD)
        .transpose(1, 0, 2).reshape(128, NKT * HD)).astype(bf16)
    wv_b = np.ascontiguousarray(
        np.asarray(Wv, np.float32).reshape(NKT, 128, HD)
        .transpose(1, 0, 2).reshape(128, NKT * HD)).astype(bf16)
    wf_b = np.asarray(Wf, np.float32).astype(bf16)
    bias_h = np.asarray(bf, np.float32).reshape(1, W)
    in_maps = []
    for core in range(8):
        b, qc = core // 4, core % 4
        if qc == 0:
            x_kv = np.concatenate([np.zeros((WIN, W), np.float32), x[b, :TQ]], 0)
            invc_h = np.maximum(0, (WIN - 1) - np.arange(TQ)).astype(np.float32)
        else:
            x_kv = x[b, (qc - 1) * TQ:(qc + 1) * TQ]
            invc_h = np.zeros(TQ, np.float32)
        pos_kv = ((qc - 1) * TQ + np.arange(TKV)).astype(np.float32)
        sinu = pos_kv[None, :] * inv_ts[:, None]        # [32, TKV]
        cos_h = np.cos(sinu).astype(np.float32)
        sin_h = np.sin(sinu).astype(np.float32)
        ccf_h = np.concatenate([cos_h, cos_h], 0)       # [64, TKV]
        snsf_h = np.concatenate([-sin_h, sin_h], 0)     # [64, TKV]
        in_maps.append({
            "x_kv": np.ascontiguousarray(x_kv).astype(bf16),
            "wq": wq_b, "wk": wk_b, "wv": wv_b, "wf": wf_b,
            "bias": bias_h,
            "ccf_t": ccf_h, "snsf_t": snsf_h,
            "m0": m0_h, "m8": m8_h,
            "invc": invc_h.reshape(NQT, 128).T.copy(),
        })
    return in_maps


def kernel(x, Wq, Wk, Wv, Wf, bf, segment_pos, _trace=False):
    nc = _get_nc()
    in_maps = make_in_maps(x, Wq, Wk, Wv, Wf, bf, segment_pos)
    res = run_bass_kernel_spmd(nc, in_maps, list(range(8)), trace=_trace)
    outs = res.results
    full = np.zeros((B, T, W), np.float32)
    for core in range(8):
        b, qc = core // 4, core % 4
        full[b, qc * TQ:(qc + 1) * TQ] = outs[core]["out"]
    if _trace:
        return full, res
    return full
